# revision 5
# baseline (speedup 1.0000x reference)
"""Trainium2 Bass kernel for nn_MultiHeadAttention_53017076301867.

Strategy (8 cores, tensor-parallel over H=16 heads, 2 heads/core):
  - Host pre-shards: per-core QKV weight column slices, W_proj row slices,
    x transposed to [E, S] bf16 (layout prep), plus mask-derived rows.
  - Each core computes its 2 heads' global causal attention + the (tiny,
    restructured) local windowed branch + a partial output projection
    over its 128 ctx features.
  - Host sums the 8 partial projections (the "all-reduce") + b_proj.

Math restructure of the reference local branch (validated to 5e-7 rel):
  - chunk-mean-then-mask == mask(mean); exp(-1.25e8) == 0 -> zero masked.
  - softmax rows q<WIN see 256 real scores + (S-WIN) zero-scores
    -> Z = sum(expS) + (S-WIN); numerator += sum_{k>=WIN} vloc[k].
  - rows q>=WIN are uniform -> mean of vloc rows.
All attention is computed in transposed form ctx^T[feat, q] so the final
projection (contract over feat) needs no transposes; per-q normalization
and global/local mask blending are folded into rank-1 PE broadcasts.
"""

import numpy as np
import ml_dtypes

S, E, H, WIN, D = 2048, 1024, 16, 256, 64
C = S // WIN            # 8 chunks
NCORES = 8
SCALE = 1.0 / (D ** 0.5)  # 0.125
BF = ml_dtypes.bfloat16

_prog_cache = {}


def build_program():
    from contextlib import ExitStack
    import concourse.tile as tile
    import concourse.mybir as mybir
    from concourse import bacc
    from concourse.masks import make_identity

    dt = mybir.dt
    f32, bf = dt.float32, dt.bfloat16
    AF = mybir.ActivationFunctionType
    ALU = mybir.AluOpType

    nc = bacc.Bacc("TRN2", target_bir_lowering=False, debug=False)

    xT = nc.dram_tensor("xT", [E, S], bf, kind="ExternalInput").ap()
    xTl = nc.dram_tensor("xTl", [E, 2 * 128], bf, kind="ExternalInput").ap()
    wqkv = nc.dram_tensor("wqkv", [E, 3, 128], bf, kind="ExternalInput").ap()
    wloc = nc.dram_tensor("wloc", [E, 3, E], bf, kind="ExternalInput").ap()
    wpr = nc.dram_tensor("wpr", [128, E], bf, kind="ExternalInput").ap()
    mrow = nc.dram_tensor("mrow", [1, S], f32, kind="ExternalInput").ap()
    wcr = nc.dram_tensor("wcr", [1, S], f32, kind="ExternalInput").ap()
    wbr = nc.dram_tensor("wbr", [1, WIN], f32, kind="ExternalInput").ap()
    outp = nc.dram_tensor("outp", [S, E], f32, kind="ExternalOutput").ap()

    with tile.TileContext(nc) as tc, ExitStack() as ctx:
        P = ctx.enter_context(tc.tile_pool(name="persist", bufs=1))

        xT_sb = P.tile([128, 8, S], bf)
        nc.sync.dma_start(out=xT_sb, in_=xT.rearrange("(c p) s -> p c s", p=128))
        xTl_sb = P.tile([128, 8, 256], bf)
        nc.sync.dma_start(out=xTl_sb, in_=xTl.rearrange("(c p) s -> p c s", p=128))
        wqkv_sb = P.tile([128, 8, 3, 128], bf)
        nc.sync.dma_start(out=wqkv_sb, in_=wqkv.rearrange("(c p) t d -> p c t d", p=128))
        wloc_sb = P.tile([128, 8, 3, E], bf)
        nc.sync.dma_start(out=wloc_sb, in_=wloc.rearrange("(c p) t e -> p c t e", p=128))
        wpr_sb = P.tile([128, E], bf)
        nc.sync.dma_start(out=wpr_sb, in_=wpr)
        mrow_sb = P.tile([1, S], f32)
        nc.sync.dma_start(out=mrow_sb, in_=mrow)
        wcr_sb = P.tile([1, S], f32)
        nc.sync.dma_start(out=wcr_sb, in_=wcr)
        wbr_sb = P.tile([1, WIN], f32)
        nc.sync.dma_start(out=wbr_sb, in_=wbr)

        ones64f = P.tile([1, 64], f32)
        nc.vector.memset(ones64f, 1.0)
        identb = P.tile([64, 64], bf)
        make_identity(nc, identb)
        identf = P.tile([64, 64], f32)
        make_identity(nc, identf)
        onesrow = P.tile([128, WIN], bf)
        nc.vector.memset(onesrow, 0.0)
        nc.vector.memset(onesrow[0:1, :], 1.0)

        QT2 = P.tile([128, S], bf)       # Q^T, rows = 2 heads x 64 dims
        KT2 = P.tile([128, S], bf)
        V2e = P.tile([128, 16, 2, 65], bf)  # [k-part, k-tile, head, d|ones]
        LT = P.tile([128, 3, 8, 256], bf)   # local Lq/Lk/Lv transposed
        QP = P.tile([64, 2, 8, 16, 16], bf)  # [d, head, u, j, i]
        KP = P.tile([64, 2, 8, 16, 16], bf)
        VP = P.tile([64, 2, 8, 16, 16], bf)
        vloc256e = P.tile([128, 2, 2, 65], bf)  # [k-part, head, k-tile, d|ones]
        tail_lhs = P.tile([128, 2, 65], bf)
        vbar = P.tile([1, 2, 64], f32)
        bloc = P.tile([128, WIN], f32)  # rows hh*64..hh*64+64 = head hh B-term
        ctxT = P.tile([128, S], bf)

        # ---------------- phase 1: global QKV projections ----------------
        with tc.tile_pool(name="ps1", bufs=3, space="PSUM") as ps1:
            for tsel, dest in ((0, QT2), (1, KT2)):
                for g in range(4):
                    ps = ps1.tile([128, 512], f32, tag="qk")
                    for ec in range(8):
                        nc.tensor.matmul(
                            ps, lhsT=wqkv_sb[:, ec, tsel, :],
                            rhs=xT_sb[:, ec, g * 512:(g + 1) * 512],
                            start=(ec == 0), stop=(ec == 7))
                    nc.any.tensor_copy(dest[:, g * 512:(g + 1) * 512], ps)
            for st in range(16):
                ps = ps1.tile([128, 128], f32, tag="v")
                for ec in range(8):
                    nc.tensor.matmul(
                        ps, lhsT=xT_sb[:, ec, st * 128:(st + 1) * 128],
                        rhs=wqkv_sb[:, ec, 2, :],
                        start=(ec == 0), stop=(ec == 7))
                nc.any.tensor_copy(
                    V2e[:, st, :, 0:64], ps.rearrange("p (h d) -> p h d", h=2))
            nc.vector.memset(V2e[:, :, :, 64], 1.0)

        # ---------------- phase 2: local (windowed) branch ----------------
        with tc.tile_pool(name="ps2", bufs=2, space="PSUM") as ps2, \
                tc.tile_pool(name="sb2", bufs=3) as sb2:
            for tsel in range(3):
                for mm in range(8):
                    ps = ps2.tile([128, 256], f32, tag="l1")
                    for ec in range(8):
                        nc.tensor.matmul(
                            ps, lhsT=wloc_sb[:, ec, tsel, mm * 128:(mm + 1) * 128],
                            rhs=xTl_sb[:, ec, :],
                            start=(ec == 0), stop=(ec == 7))
                    nc.any.tensor_copy(LT[:, tsel, mm, :], ps)

            # permuted layouts: dest[d, h, u, j, i] = L[h*128 + u*16 + j, i*64 + d]
            for tsel, dest in ((0, QP), (1, KP), (2, VP)):
                for hh in range(2):
                    for i in range(16):
                        src = LT[(i % 2) * 64:(i % 2) * 64 + 64, tsel, i // 2,
                                 hh * 128:(hh + 1) * 128]
                        nc.sync.dma_start(
                            out=dest[:, hh, :, :, i],
                            in_=src.rearrange("d (u j) -> d u j", u=8))

            for hh in range(2):
                # tail/all sums of vloc rows (d on partitions), then to rows
                tcol = sb2.tile([64, 1], f32, tag="tcol")
                nc.vector.reduce_sum(tcol, VP[:, hh, 1:8, :, :],
                                     axis=mybir.AxisListType.XYZ)
                vallc = sb2.tile([64, 1], f32, tag="vallc")
                nc.vector.reduce_sum(vallc, VP[:, hh, :, :, :],
                                     axis=mybir.AxisListType.XYZ)
                tcolb = sb2.tile([64, 1], bf, tag="tcolb")
                nc.vector.tensor_copy(tcolb, tcol)
                pst = ps2.tile([1, 64], bf, tag="misc", bufs=1)
                nc.tensor.transpose(pst, tcolb, identb)
                nc.vector.memset(tail_lhs[:, hh, :], 0.0)
                nc.vector.tensor_copy(tail_lhs[0:1, hh, 0:64], pst)
                nc.vector.memset(tail_lhs[0:1, hh, 64:65], float(S - WIN))
                psv = ps2.tile([1, 64], f32, tag="misc", bufs=1)
                nc.tensor.transpose(psv, vallc, identf)
                nc.vector.tensor_copy(vbar[:, hh, :], psv)

                for kt in range(2):
                    pst2 = ps2.tile([128, 64], bf, tag="misc", bufs=1)
                    nc.tensor.transpose(
                        pst2, VP[:, hh, 0, kt * 8:(kt + 1) * 8, :], identb)
                    nc.vector.tensor_copy(vloc256e[:, hh, kt, 0:64], pst2)
            nc.vector.memset(vloc256e[:, :, :, 64], 1.0)

            for hh in range(2):
                ploc = ps2.tile([65, WIN], f32, tag="pvloc")
                for kt in range(2):
                    sps = ps2.tile([128, WIN], f32, tag="sloc")
                    for u in range(8):
                        nc.tensor.matmul(
                            sps, lhsT=KP[:, hh, u, kt * 8:(kt + 1) * 8, :],
                            rhs=QP[:, hh, u, :, :],
                            start=(u == 0), stop=(u == 7))
                    et = sb2.tile([128, WIN], bf, tag="eloc")
                    nc.scalar.activation(et, sps, AF.Exp, scale=SCALE / C)
                    nc.gpsimd.affine_select(
                        et, et, pattern=[[1, WIN]], base=-kt * 128,
                        channel_multiplier=-1, compare_op=ALU.is_ge, fill=0.0)
                    nc.tensor.matmul(ploc, lhsT=vloc256e[:, hh, kt, :], rhs=et,
                                     start=(kt == 0), stop=False,
                                     skip_group_check=True)
                nc.tensor.matmul(ploc, lhsT=tail_lhs[:, hh, :], rhs=onesrow,
                                 start=False, stop=True, skip_group_check=True)
                zl = sb2.tile([1, WIN], f32, tag="zl")
                nc.vector.reciprocal(zl, ploc[64:65, :])
                rbl = sb2.tile([1, WIN], f32, tag="rbl")
                nc.vector.tensor_mul(rbl, zl, wbr_sb)
                rblp = ps2.tile([64, WIN], f32, tag="misc", bufs=1)
                nc.tensor.matmul(rblp, lhsT=ones64f, rhs=rbl, start=True, stop=True)
                rbls = sb2.tile([64, WIN], f32, tag="rbls")
                nc.any.tensor_copy(rbls, rblp)
                nc.vector.tensor_mul(bloc[hh * 64:(hh + 1) * 64, :],
                                     ploc[0:64, :], rbls)

        # ---------------- phase 3: global causal attention ----------------
        with tc.tile_pool(name="ps3", bufs=3, space="PSUM") as ps3, \
                tc.tile_pool(name="sb3", bufs=4) as sb3:
            for hh in range(2):
                hs = slice(hh * 64, hh * 64 + 64)
                for g in range(4):
                    gps = ps3.tile([65, 512], f32, tag="gctx", bufs=2)
                    nkt = 4 * g + 4
                    for t in range(nkt):
                        sps = ps3.tile([128, 512], f32, tag="sT", bufs=3)
                        nc.tensor.matmul(
                            sps, lhsT=KT2[hs, t * 128:(t + 1) * 128],
                            rhs=QT2[hs, g * 512:(g + 1) * 512],
                            start=True, stop=True)
                        et = sb3.tile([128, 512], bf, tag="expT")
                        nc.scalar.activation(et, sps, AF.Exp, scale=SCALE)
                        if t >= 4 * g:
                            nc.gpsimd.affine_select(
                                et, et, pattern=[[1, 512]],
                                base=g * 512 - t * 128, channel_multiplier=-1,
                                compare_op=ALU.is_ge, fill=0.0)
                        nc.tensor.matmul(gps, lhsT=V2e[:, t, hh, :], rhs=et,
                                         start=(t == 0), stop=(t == nkt - 1),
                                         skip_group_check=True)
                    zrow = sb3.tile([1, 512], f32, tag="zrow")
                    nc.vector.reciprocal(zrow, gps[64:65, :])
                    ra = sb3.tile([1, 512], f32, tag="ra")
                    nc.vector.tensor_mul(ra, zrow, mrow_sb[:, g * 512:(g + 1) * 512])
                    rbp = ps3.tile([64, 512], f32, tag="small", bufs=2)
                    nc.tensor.matmul(rbp, lhsT=ones64f, rhs=ra, start=True, stop=True)
                    rbs = sb3.tile([64, 512], f32, tag="rbs")
                    nc.any.tensor_copy(rbs, rbp)
                    cps = ps3.tile([64, 512], f32, tag="small", bufs=2)
                    nc.tensor.matmul(cps, lhsT=vbar[:, hh, :],
                                     rhs=wcr_sb[:, g * 512:(g + 1) * 512],
                                     start=True, stop=True)
                    dst = ctxT[hs, g * 512:(g + 1) * 512]
                    nc.vector.tensor_mul(dst, gps[0:64, :], rbs)
                    nc.vector.tensor_add(dst, dst, cps)
                    if g == 0:
                        nc.vector.tensor_add(ctxT[hs, 0:WIN], ctxT[hs, 0:WIN],
                                             bloc[hs, :])

        # ---------------- phase 4: output projection (partial) ----------------
        with tc.tile_pool(name="ps4", bufs=4, space="PSUM") as ps4, \
                tc.tile_pool(name="sb4", bufs=4) as sb4:
            for qt in range(16):
                for half in range(2):
                    pp = ps4.tile([128, 512], f32, tag="pp")
                    nc.tensor.matmul(
                        pp, lhsT=ctxT[:, qt * 128:(qt + 1) * 128],
                        rhs=wpr_sb[:, half * 512:(half + 1) * 512],
                        start=True, stop=True)
                    ot = sb4.tile([128, 512], f32, tag="ot")
                    nc.any.tensor_copy(ot, pp)
                    nc.sync.dma_start(
                        out=outp[qt * 128:(qt + 1) * 128,
                                 half * 512:(half + 1) * 512],
                        in_=ot)

    nc.compile()
    return nc


def prep_inputs(x, global_attention_mask, W_local_query, W_local_key,
                W_local_value, W_query, W_key, W_value, W_proj):
    """Host-side sharding/layout prep. Returns list of per-core input dicts."""
    def b(a):
        return np.ascontiguousarray(np.asarray(a, np.float32)).astype(BF)

    x2 = np.asarray(x, np.float32).reshape(S, E)
    xT_np = np.ascontiguousarray(x2.T).astype(BF)                   # [E, S]
    wloc_np = np.ascontiguousarray(
        np.stack([np.asarray(W_local_query, np.float32),
                  np.asarray(W_local_key, np.float32),
                  np.asarray(W_local_value, np.float32)], axis=1)).astype(BF)
    m = np.asarray(global_attention_mask, np.float32).reshape(S)
    q = np.arange(S)
    mrow_np = np.ascontiguousarray(m.reshape(1, S))
    wcr_np = np.ascontiguousarray(((1.0 - m) * (q >= WIN) / S).reshape(1, S)
                                  ).astype(np.float32)
    wbr_np = np.ascontiguousarray(((1.0 - m)[:WIN]).reshape(1, WIN)
                                  ).astype(np.float32)
    Wq = np.asarray(W_query, np.float32)
    Wk = np.asarray(W_key, np.float32)
    Wv = np.asarray(W_value, np.float32)
    Wp = np.asarray(W_proj, np.float32)

    in_maps = []
    for i in range(NCORES):
        cs = slice(i * 128, (i + 1) * 128)
        in_maps.append({
            "xT": xT_np,
            "xTl": np.ascontiguousarray(xT_np[:, i * 256:(i + 1) * 256]),
            "wqkv": b(np.stack([Wq[:, cs], Wk[:, cs], Wv[:, cs]], axis=1)),
            "wloc": wloc_np,
            "wpr": b(Wp[cs, :]),
            "mrow": mrow_np,
            "wcr": wcr_np,
            "wbr": wbr_np,
        })
    return in_maps


def kernel(x, global_attention_mask, W_local_query, W_local_key, W_local_value,
           W_query, W_key, W_value, W_proj, b_proj):
    from concourse.bass_utils import run_bass_kernel_spmd

    if "nc" not in _prog_cache:
        _prog_cache["nc"] = build_program()
    nc = _prog_cache["nc"]

    in_maps = prep_inputs(x, global_attention_mask, W_local_query, W_local_key,
                          W_local_value, W_query, W_key, W_value, W_proj)
    res = run_bass_kernel_spmd(nc, in_maps, core_ids=list(range(NCORES)))
    out = np.zeros((S, E), np.float32)
    for r in res.results:
        out += r["outp"]
    out = out + np.asarray(b_proj, np.float32)[None, :]
    return out[None].astype(np.float32)


# revision 15
# speedup vs baseline: 4.3853x; 4.3853x over previous
"""Trainium2 Bass kernel for nn_MultiHeadAttention_53017076301867.

Strategy (8 cores, tensor-parallel over H=16 heads, 2 heads/core):
  - Host pre-shards: per-core QKV weight column slices, W_proj row slices,
    x transposed to [E, S] bf16 (layout prep), plus mask-derived rows.
  - Each core computes its 2 heads' global causal attention + the (tiny,
    restructured) local windowed branch + a partial output projection
    over its 128 ctx features.
  - Host sums the 8 partial projections (the "all-reduce") + b_proj.

Math restructure of the reference local branch (validated to 5e-7 rel):
  - chunk-mean-then-mask == mask(mean); exp(-1.25e8) == 0 -> zero masked.
  - softmax rows q<WIN see 256 real scores + (S-WIN) zero-scores
    -> Z = sum(expS) + (S-WIN); numerator += sum_{k>=WIN} vloc[k].
  - rows q>=WIN are uniform -> mean of vloc rows.
All attention is computed in transposed form ctx^T[feat, q] so the final
projection (contract over feat) needs no transposes; per-q normalization
and global/local mask blending are folded into rank-1 PE broadcasts.
"""

import numpy as np
import ml_dtypes

S, E, H, WIN, D = 2048, 1024, 16, 256, 64
C = S // WIN            # 8 chunks
NCORES = 8
SCALE = 1.0 / (D ** 0.5)  # 0.125
BF = ml_dtypes.bfloat16

_prog_cache = {}


def build_program():
    from contextlib import ExitStack
    import concourse.tile as tile
    import concourse.mybir as mybir
    from concourse import bacc
    from concourse.masks import make_identity

    dt = mybir.dt
    f32, bf = dt.float32, dt.bfloat16
    AF = mybir.ActivationFunctionType
    ALU = mybir.AluOpType

    nc = bacc.Bacc("TRN2", target_bir_lowering=False, debug=False)

    xT = nc.dram_tensor("xT", [E, S], bf, kind="ExternalInput").ap()
    xTl = nc.dram_tensor("xTl", [E, 2 * 128], bf, kind="ExternalInput").ap()
    wqkv = nc.dram_tensor("wqkv", [E, 3, 128], bf, kind="ExternalInput").ap()
    wloc = nc.dram_tensor("wloc", [E, 3, E], bf, kind="ExternalInput").ap()
    wpr = nc.dram_tensor("wpr", [128, E], bf, kind="ExternalInput").ap()
    mrow = nc.dram_tensor("mrow", [1, S], f32, kind="ExternalInput").ap()
    wcr = nc.dram_tensor("wcr", [1, S], f32, kind="ExternalInput").ap()
    wbr = nc.dram_tensor("wbr", [1, WIN], f32, kind="ExternalInput").ap()
    outp = nc.dram_tensor("outp", [S, E], f32, kind="ExternalOutput").ap()

    with tile.TileContext(nc) as tc, ExitStack() as ctx:
        P = ctx.enter_context(tc.tile_pool(name="persist", bufs=1))

        xT_sb = P.tile([128, 8, S], bf)
        nc.sync.dma_start(out=xT_sb, in_=xT.rearrange("(c p) s -> p c s", p=128))
        xTl_sb = P.tile([128, 8, 256], bf)
        nc.sync.dma_start(out=xTl_sb, in_=xTl.rearrange("(c p) s -> p c s", p=128))
        wqkv_sb = P.tile([128, 8, 3, 128], bf)
        nc.sync.dma_start(out=wqkv_sb, in_=wqkv.rearrange("(c p) t d -> p c t d", p=128))
        wloc_sb = P.tile([128, 8, 3, E], bf)
        nc.sync.dma_start(out=wloc_sb, in_=wloc.rearrange("(c p) t e -> p c t e", p=128))
        wpr_sb = P.tile([128, E], bf)
        nc.sync.dma_start(out=wpr_sb, in_=wpr)
        mrow_sb = P.tile([1, S], f32)
        nc.sync.dma_start(out=mrow_sb, in_=mrow)
        wcr_sb = P.tile([1, S], f32)
        nc.sync.dma_start(out=wcr_sb, in_=wcr)
        wbr_sb = P.tile([1, WIN], f32)
        nc.sync.dma_start(out=wbr_sb, in_=wbr)

        ones64f = P.tile([1, 64], f32)
        nc.vector.memset(ones64f, 1.0)
        identb = P.tile([64, 64], bf)
        make_identity(nc, identb)
        identf = P.tile([64, 64], f32)
        make_identity(nc, identf)
        onesrow = P.tile([128, WIN], bf)
        nc.vector.memset(onesrow, 0.0)
        nc.vector.memset(onesrow[0:1, :], 1.0)

        QT2 = P.tile([128, S], bf)       # Q^T, rows = 2 heads x 64 dims
        KT2 = P.tile([128, S], bf)
        V2e = P.tile([128, 16, 2, 65], bf)  # [k-part, k-tile, head, d|ones]
        QP = P.tile([64, 2, 8, 16, 16], bf)  # [d, head, u, j, i]
        KP = P.tile([64, 2, 8, 16, 16], bf)
        VP = P.tile([64, 2, 8, 16, 16], bf)
        vloc256e = P.tile([128, 2, 2, 65], bf)  # [k-part, head, k-tile, d|ones]
        tail_lhs = P.tile([128, 2, 65], bf)
        vbar = P.tile([1, 2, 64], f32)
        bloc = P.tile([128, WIN], f32)  # rows hh*64..hh*64+64 = head hh B-term
        ctxT = P.tile([128, S], bf)

        # ---------------- phase 1: global QKV projections ----------------
        with tc.tile_pool(name="ps1", bufs=3, space="PSUM") as ps1:
            for tsel, dest in ((0, QT2), (1, KT2)):
                for g in range(4):
                    ps = ps1.tile([128, 512], f32, tag="qk")
                    for ec in range(8):
                        nc.tensor.matmul(
                            ps, lhsT=wqkv_sb[:, ec, tsel, :],
                            rhs=xT_sb[:, ec, g * 512:(g + 1) * 512],
                            start=(ec == 0), stop=(ec == 7))
                    nc.any.tensor_copy(dest[:, g * 512:(g + 1) * 512], ps)
            for st in range(16):
                ps = ps1.tile([128, 128], f32, tag="v")
                for ec in range(8):
                    nc.tensor.matmul(
                        ps, lhsT=xT_sb[:, ec, st * 128:(st + 1) * 128],
                        rhs=wqkv_sb[:, ec, 2, :],
                        start=(ec == 0), stop=(ec == 7))
                nc.any.tensor_copy(
                    V2e[:, st, :, 0:64], ps.rearrange("p (h d) -> p h d", h=2))
            nc.vector.memset(V2e[:, :, :, 64], 1.0)

        # ---------------- phase 2: local (windowed) branch ----------------
        with tc.tile_pool(name="ps2", bufs=2, space="PSUM") as ps2, \
                tc.tile_pool(name="sb2", bufs=3) as sb2:
            # permuted local projections, directly from matmuls:
            # dest[d, h, i, u, j] = L[h*128 + u*16 + j, i*64 + d]
            for tsel, dest in ((0, QP), (1, KP), (2, VP)):
                for i in range(16):
                    ps = ps2.tile([64, 256], f32, tag="l1")
                    for ec in range(8):
                        nc.tensor.matmul(
                            ps, lhsT=wloc_sb[:, ec, tsel, i * 64:(i + 1) * 64],
                            rhs=xTl_sb[:, ec, :],
                            start=(ec == 0), stop=(ec == 7))
                    nc.any.tensor_copy(
                        dest[:, :, :, :, i],
                        ps.rearrange("d (h u j) -> d h u j", h=2, u=8))

            for hh in range(2):
                # tail/all sums of vloc rows (d on partitions), then to rows
                tcol = sb2.tile([64, 1], f32, tag="tcol")
                nc.vector.reduce_sum(tcol, VP[:, hh, 1:8, :, :],
                                     axis=mybir.AxisListType.XYZ)
                vallc = sb2.tile([64, 1], f32, tag="vallc")
                nc.vector.reduce_sum(vallc, VP[:, hh, :, :, :],
                                     axis=mybir.AxisListType.XYZ)
                tcolb = sb2.tile([64, 1], bf, tag="tcolb")
                nc.vector.tensor_copy(tcolb, tcol)
                pst = ps2.tile([1, 64], bf, tag="misc", bufs=1)
                nc.tensor.transpose(pst, tcolb, identb)
                nc.vector.memset(tail_lhs[:, hh, :], 0.0)
                nc.vector.tensor_copy(tail_lhs[0:1, hh, 0:64], pst)
                nc.vector.memset(tail_lhs[0:1, hh, 64:65], float(S - WIN))
                psv = ps2.tile([1, 64], f32, tag="misc", bufs=1)
                nc.tensor.transpose(psv, vallc, identf)
                nc.vector.tensor_copy(vbar[:, hh, :], psv)

                for kt in range(2):
                    pst2 = ps2.tile([128, 64], bf, tag="misc", bufs=1)
                    nc.tensor.transpose(
                        pst2, VP[:, hh, 0, kt * 8:(kt + 1) * 8, :], identb)
                    nc.vector.tensor_copy(vloc256e[:, hh, kt, 0:64], pst2)
            nc.vector.memset(vloc256e[:, :, :, 64], 1.0)

            for hh in range(2):
                ploc = ps2.tile([65, WIN], f32, tag="pvloc")
                for kt in range(2):
                    sps = ps2.tile([128, WIN], f32, tag="sloc")
                    for u in range(8):
                        nc.tensor.matmul(
                            sps, lhsT=KP[:, hh, u, kt * 8:(kt + 1) * 8, :],
                            rhs=QP[:, hh, u, :, :],
                            start=(u == 0), stop=(u == 7))
                    et = sb2.tile([128, WIN], bf, tag="eloc")
                    nc.scalar.activation(et, sps, AF.Exp, scale=SCALE / C)
                    nc.gpsimd.affine_select(
                        et, et, pattern=[[1, WIN]], base=-kt * 128,
                        channel_multiplier=-1, compare_op=ALU.is_ge, fill=0.0)
                    nc.tensor.matmul(ploc, lhsT=vloc256e[:, hh, kt, :], rhs=et,
                                     start=(kt == 0), stop=False,
                                     skip_group_check=True)
                nc.tensor.matmul(ploc, lhsT=tail_lhs[:, hh, :], rhs=onesrow,
                                 start=False, stop=True, skip_group_check=True)
                zl = sb2.tile([1, WIN], f32, tag="zl")
                nc.vector.reciprocal(zl, ploc[64:65, :])
                rbl = sb2.tile([1, WIN], f32, tag="rbl")
                nc.vector.tensor_mul(rbl, zl, wbr_sb)
                rblp = ps2.tile([64, WIN], f32, tag="misc", bufs=1)
                nc.tensor.matmul(rblp, lhsT=ones64f, rhs=rbl, start=True, stop=True)
                rbls = sb2.tile([64, WIN], f32, tag="rbls")
                nc.any.tensor_copy(rbls, rblp)
                nc.vector.tensor_mul(bloc[hh * 64:(hh + 1) * 64, :],
                                     ploc[0:64, :], rbls)

        # ---------------- phase 3: global causal attention ----------------
        with tc.tile_pool(name="ps3", bufs=3, space="PSUM") as ps3, \
                tc.tile_pool(name="sb3", bufs=4) as sb3:
            for hh in range(2):
                hs = slice(hh * 64, hh * 64 + 64)
                for g in range(4):
                    gps = ps3.tile([65, 512], f32, tag="gctx", bufs=2)
                    nkt = 4 * g + 4
                    for t in range(nkt):
                        sps = ps3.tile([128, 512], f32, tag="sT", bufs=3)
                        nc.tensor.matmul(
                            sps, lhsT=KT2[hs, t * 128:(t + 1) * 128],
                            rhs=QT2[hs, g * 512:(g + 1) * 512],
                            start=True, stop=True)
                        et = sb3.tile([128, 512], bf, tag="expT")
                        nc.scalar.activation(et, sps, AF.Exp, scale=SCALE)
                        if t >= 4 * g:
                            nc.gpsimd.affine_select(
                                et, et, pattern=[[1, 512]],
                                base=g * 512 - t * 128, channel_multiplier=-1,
                                compare_op=ALU.is_ge, fill=0.0)
                        nc.tensor.matmul(gps, lhsT=V2e[:, t, hh, :], rhs=et,
                                         start=(t == 0), stop=(t == nkt - 1),
                                         skip_group_check=True)
                    zrow = sb3.tile([1, 512], f32, tag="zrow")
                    nc.vector.reciprocal(zrow, gps[64:65, :])
                    ra = sb3.tile([1, 512], f32, tag="ra")
                    nc.vector.tensor_mul(ra, zrow, mrow_sb[:, g * 512:(g + 1) * 512])
                    rbp = ps3.tile([64, 512], f32, tag="small", bufs=2)
                    nc.tensor.matmul(rbp, lhsT=ones64f, rhs=ra, start=True, stop=True)
                    rbs = sb3.tile([64, 512], f32, tag="rbs")
                    nc.any.tensor_copy(rbs, rbp)
                    cps = ps3.tile([64, 512], f32, tag="small", bufs=2)
                    nc.tensor.matmul(cps, lhsT=vbar[:, hh, :],
                                     rhs=wcr_sb[:, g * 512:(g + 1) * 512],
                                     start=True, stop=True)
                    dst = ctxT[hs, g * 512:(g + 1) * 512]
                    nc.vector.tensor_mul(dst, gps[0:64, :], rbs)
                    nc.vector.tensor_add(dst, dst, cps)
                    if g == 0:
                        nc.vector.tensor_add(ctxT[hs, 0:WIN], ctxT[hs, 0:WIN],
                                             bloc[hs, :])

        # ---------------- phase 4: output projection (partial) ----------------
        with tc.tile_pool(name="ps4", bufs=4, space="PSUM") as ps4, \
                tc.tile_pool(name="sb4", bufs=4) as sb4:
            for qt in range(16):
                for half in range(2):
                    pp = ps4.tile([128, 512], f32, tag="pp")
                    nc.tensor.matmul(
                        pp, lhsT=ctxT[:, qt * 128:(qt + 1) * 128],
                        rhs=wpr_sb[:, half * 512:(half + 1) * 512],
                        start=True, stop=True)
                    ot = sb4.tile([128, 512], f32, tag="ot")
                    nc.any.tensor_copy(ot, pp)
                    nc.sync.dma_start(
                        out=outp[qt * 128:(qt + 1) * 128,
                                 half * 512:(half + 1) * 512],
                        in_=ot)

    nc.compile()
    return nc


def prep_inputs(x, global_attention_mask, W_local_query, W_local_key,
                W_local_value, W_query, W_key, W_value, W_proj):
    """Host-side sharding/layout prep. Returns list of per-core input dicts."""
    def b(a):
        return np.ascontiguousarray(np.asarray(a, np.float32)).astype(BF)

    x2 = np.asarray(x, np.float32).reshape(S, E)
    xT_np = np.ascontiguousarray(x2.T).astype(BF)                   # [E, S]
    wloc_np = np.ascontiguousarray(
        np.stack([np.asarray(W_local_query, np.float32),
                  np.asarray(W_local_key, np.float32),
                  np.asarray(W_local_value, np.float32)], axis=1)).astype(BF)
    m = np.asarray(global_attention_mask, np.float32).reshape(S)
    q = np.arange(S)
    mrow_np = np.ascontiguousarray(m.reshape(1, S))
    wcr_np = np.ascontiguousarray(((1.0 - m) * (q >= WIN) / S).reshape(1, S)
                                  ).astype(np.float32)
    wbr_np = np.ascontiguousarray(((1.0 - m)[:WIN]).reshape(1, WIN)
                                  ).astype(np.float32)
    Wq = np.asarray(W_query, np.float32)
    Wk = np.asarray(W_key, np.float32)
    Wv = np.asarray(W_value, np.float32)
    Wp = np.asarray(W_proj, np.float32)

    in_maps = []
    for i in range(NCORES):
        cs = slice(i * 128, (i + 1) * 128)
        in_maps.append({
            "xT": xT_np,
            "xTl": np.ascontiguousarray(xT_np[:, i * 256:(i + 1) * 256]),
            "wqkv": b(np.stack([Wq[:, cs], Wk[:, cs], Wv[:, cs]], axis=1)),
            "wloc": wloc_np,
            "wpr": b(Wp[cs, :]),
            "mrow": mrow_np,
            "wcr": wcr_np,
            "wbr": wbr_np,
        })
    return in_maps


def kernel(x, global_attention_mask, W_local_query, W_local_key, W_local_value,
           W_query, W_key, W_value, W_proj, b_proj):
    from concourse.bass_utils import run_bass_kernel_spmd

    if "nc" not in _prog_cache:
        _prog_cache["nc"] = build_program()
    nc = _prog_cache["nc"]

    in_maps = prep_inputs(x, global_attention_mask, W_local_query, W_local_key,
                          W_local_value, W_query, W_key, W_value, W_proj)
    res = run_bass_kernel_spmd(nc, in_maps, core_ids=list(range(NCORES)))
    out = np.zeros((S, E), np.float32)
    for r in res.results:
        out += r["outp"]
    out = out + np.asarray(b_proj, np.float32)[None, :]
    return out[None].astype(np.float32)
